# revision 25
# baseline (speedup 1.0000x reference)
"""Trainium2 Bass kernel for nn_AdjacencyMaskedNet.

Reference math (N=4096, I=512, O=512, O_=8 groups, H=2048, GROUP=64):
    for each group g: h_g = relu((x * A_mask[:, g]) @ W1 + b1)
                      y_g = h_g @ W2 + b2
    out[n, j] = y_{col_idx[j]}[n, j]

With the canonical inputs, A_mask[:, g] is the prefix mask over the first
64*(g+1) inputs and col_idx = arange(512) // 64.  Structural wins:

  1. mm1 is computed *incrementally* in PSUM: a_g = a_{g-1} + (next 64-col
     block of x) @ (matching W1 rows).  One full-matmul worth of FLOPs
     instead of 8.
  2. mm2 only needs the 64 output columns of each group's y: an 8x cut.
  3. The 8 running-sum snapshots per H-block are produced by TWO chains of
     K=128 increments (K=64 LDWEIGHTS from sliced tiles measured 3x slower
     and un-hidden):
        odd  chain: B0B1 | B2B3 | B4B5 | B6B7      -> a1, a3, a5, a7
        even chain: B0 | B1B2 | B3B4 | B5B6        -> a0, a2, a4, a6
     where the even chain's pair blocks come from a 64-row-shifted DMA view
     of the same W1/x DRAM.
  4. mm2 (M=64) instructions alternate PE column halves (even groups ->
     cols 0-63, odd -> 64-127), so consecutive mm2s run concurrently in
     the array (2x), packing two groups per PSUM bank.

Sharding: data-parallel over batch, 512 rows per core, 8 cores, no
collectives.  Each core computes outT (O, 512) for its batch shard.

dtypes: mm1 in float32r (full-rate fp32, ~1.5e-4 component error), mm2 in
bf16 (h tiles are written in bf16 directly by the relu).  PSUM accumulates
fp32.
"""

import sys

if "/opt/trn_rl_repo" not in sys.path:
    sys.path.insert(0, "/opt/trn_rl_repo")

import numpy as np
import ml_dtypes

N, I, O, O_, H = 4096, 512, 512, 8, 2048
GROUP = O // O_  # 64
NCORES = 8
NC = N // NCORES  # 512 batch rows per core
HB = H // 128  # 16 H-blocks
NPAIR = O // 128  # 4 psum banks for mm2 (two 64-col groups per bank)

_CACHE = {}


def _canonical_mask():
    g = np.arange(O_)
    return (np.arange(I)[:, None] < (g[None, :] + 1) * (I // O_)).astype(np.float32)


def _build_program(with_bias):
    """Build + compile the Bass program once per process."""
    import concourse.tile as tile
    from concourse import bacc, mybir

    f32 = mybir.dt.float32
    f32r = mybir.dt.float32r
    bf16 = mybir.dt.bfloat16
    Relu = mybir.ActivationFunctionType.Relu
    Ident = mybir.ActivationFunctionType.Identity
    Alu = mybir.AluOpType

    nc = bacc.Bacc("TRN2", target_bir_lowering=False, debug=False, num_devices=NCORES)

    xt = nc.dram_tensor("xt", [I, NC], f32, kind="ExternalInput").ap()
    w1 = nc.dram_tensor("w1", [I, H], f32, kind="ExternalInput").ap()
    w2 = nc.dram_tensor("w2", [H, O], bf16, kind="ExternalInput").ap()
    b1t = nc.dram_tensor("b1t", [128, HB], f32, kind="ExternalInput").ap()
    b2t = nc.dram_tensor("b2t", [128, NPAIR], f32, kind="ExternalInput").ap()
    ot = nc.dram_tensor("ot", [O, NC], f32, kind="ExternalOutput").ap()

    with tile.TileContext(nc) as tc:
        with (
            tc.tile_pool(name="const", bufs=1) as cp,
            tc.tile_pool(name="hpool", bufs=12) as hp,
            tc.tile_pool(name="opool", bufs=1) as op,
            tc.tile_pool(name="ps1", bufs=2, space="PSUM") as ps1,
            tc.tile_pool(name="ps2", bufs=1, space="PSUM") as ps2,
        ):
            # ---- tiles for resident inputs ----
            xnat = [cp.tile([128, NC], f32r, name=f"xn{m}") for m in range(4)]
            xsh = [cp.tile([128, NC], f32r, name=f"xh{m}") for m in range(3)]
            w1nat = [
                [cp.tile([128, 512], f32r, name=f"w1n{m}_{q}") for q in range(4)]
                for m in range(4)
            ]
            w1sh = [
                [cp.tile([128, 512], f32r, name=f"w1s{m}_{q}") for q in range(4)]
                for m in range(3)
            ]
            w2sb = [cp.tile([128, O], bf16, name=f"w2_{k}") for k in range(HB)]
            b1sb = cp.tile([128, HB], f32, name="b1sb")
            b2sb = cp.tile([128, NPAIR], f32, name="b2sb")

            # ---- DMAs in first-use order (all on the sync HWDGE ring):
            # x block then its step-matching W1 chunk, so chain (hb0, s=0)
            # can start after the first two transfers.
            def dma_w1n(m, q):
                cols = slice(q * 512, (q + 1) * 512)
                nc.sync.dma_start(
                    w1nat[m][q][:], w1[m * 128 : (m + 1) * 128, cols].bitcast(f32r)
                )

            def dma_w1s(m, q):
                # shifted view = partition-shifted copy of the natural tiles,
                # sourced on-chip (gpsimd SWDGE, SBUF->SBUF) to save 3MB of
                # HBM reads during the startup ramp
                nc.gpsimd.dma_start(w1sh[m][q][0:64, :], w1nat[m][q][64:128, :])
                nc.gpsimd.dma_start(w1sh[m][q][64:128, :], w1nat[m + 1][q][0:64, :])

            nc.sync.dma_start(xnat[0][:], xt[0:128, :].bitcast(f32r))
            dma_w1n(0, 0)
            nc.sync.dma_start(b1sb[:], b1t[:])
            nc.sync.dma_start(b2sb[:], b2t[:])
            for m in range(1, 4):
                nc.sync.dma_start(xnat[m][:], xt[m * 128 : (m + 1) * 128, :].bitcast(f32r))
                dma_w1n(m, 0)
                nc.gpsimd.dma_start(xsh[m - 1][0:64, :], xnat[m - 1][64:128, :])
                nc.gpsimd.dma_start(xsh[m - 1][64:128, :], xnat[m][0:64, :])
                dma_w1s(m - 1, 0)
            for q in range(4):
                if q > 0:
                    for m in range(4):
                        dma_w1n(m, q)
                    for m in range(3):
                        dma_w1s(m, q)
                for k in range(q * 4, (q + 1) * 4):
                    nc.sync.dma_start(w2sb[k][:], w2[k * 128 : (k + 1) * 128, :])

            # zero operands for the has_written-initializing dummy matmuls
            z1 = cp.tile([1, 128], bf16, name="z1")
            nc.gpsimd.memset(z1[:], 0.0)
            z2 = cp.tile([1, NC], bf16, name="z2")
            nc.gpsimd.memset(z2[:], 0.0)

            # ---- mm2 accumulators: 4 banks, two 64-col groups per bank.
            # start=True on any matmul clears has_written for the WHOLE bank
            # (nuking the other group), so init each bank once with a zero
            # matmul and accumulate with start=False afterwards.
            mm2ps = [ps2.tile([128, NC], f32, name=f"mm2_{t}") for t in range(NPAIR)]

            def init_mm2_banks():
                # init has_written for the whole of each mm2 bank once, so the
                # two packed groups can both accumulate with start=False
                for t in range(NPAIR):
                    nc.tensor.matmul(
                        mm2ps[t][:], z1[:], z2[:], start=True, stop=False,
                        skip_group_check=True,
                    )

            # mm1 increment operands for (parity, step, hb-quad):
            #   odd  chain step s: lhsT = w1nat[s][q] slice, rhs = xnat[s]
            #   even chain step 0: lhsT = w1nat[0][q][0:64] slice, rhs = xnat[0][0:64]
            #   even chain step s>0: lhsT = w1sh[s-1][q] slice, rhs = xsh[s-1]
            def mm1_ops(parity, s, hb):
                q, hq = hb // 4, hb % 4
                colsl = slice(hq * 128, (hq + 1) * 128)
                if parity == 1:
                    return w1nat[s][q][:, colsl], xnat[s][:]
                if s == 0:
                    return w1nat[0][q][0:64, colsl], xnat[0][0:64, :]
                return w1sh[s - 1][q][:, colsl], xsh[s - 1][:]

            # ---- main loop: hb pairs, 4 chains in flight:
            #   c0=(hbA, even) c1=(hbA, odd) c2=(hbB, even) c3=(hbB, odd)
            # mm2 runs one step behind mm1 (software pipelining) so the PE
            # never head-of-line blocks on a relu.  Chain (parity p, step s)
            # snapshots group g = 2s + p; mm2 for group g targets psum bank
            # s, column half p -- consecutive mm2s alternate PE column
            # halves and run concurrently (M=64 col tiling).
            # Each hb's even+odd chains share a double-width (128, 1024)
            # accumulator (2 banks; even chain in cols 0-511, odd in
            # 512-1023): ONE wide relu per (hb, step) snapshots both,
            # halving the relu op count.  Both halves share hb, so the
            # per-partition b1 bias column still applies in bias mode.
            # The fp32r<->bf16 dtype switch on the PE costs ~100-250ns, so
            # all 16 mm1s of an hb pair are batched, then all 16 mm2s
            # (measured 180ns/MM at batch=16 vs 244 at batch=4).  The relus
            # overlap the mm1 batch; the mm2 batch overlaps the next pair's
            # weight DMAs and relu completions.
            # PE stream per step: [4x mm1(pair P, s)] [4x mm2(pair P-1, s)].
            # The mm2 batch of the PREVIOUS pair gives the PE ~8 matmuls of
            # ready work between the dependent mm1(s) -> relu(s) -> mm1(s+1)
            # links, hiding the ~1.2us wide-relu latency.
            NSTEP = 4

            def emit_mm2(pair, hsnap, s, evac):
                for w in range(2):
                    hb = 2 * pair + w
                    for p in range(2):
                        g = 2 * s + p
                        nc.tensor.matmul(
                            mm2ps[s][64 * p : 64 * p + 64, :],
                            w2sb[hb][:, g * GROUP : (g + 1) * GROUP],
                            hsnap[w][s][:, p * NC : (p + 1) * NC],
                            start=False,
                            stop=(hb == HB - 1 and p == 1),
                            skip_group_check=True,
                        )
                if evac:
                    os_ = op.tile([128, NC], f32, name=f"os{s}")
                    if s % 2 == 0:
                        nc.scalar.activation(
                            os_[:], mm2ps[s][:], Ident, bias=b2sb[:, s : s + 1]
                        )
                    else:
                        nc.vector.tensor_scalar_add(os_[:], mm2ps[s][:], b2sb[:, s : s + 1])
                    nc.sync.dma_start(ot[s * 128 : (s + 1) * 128, :], os_[:])

            def emit_relu(h, acc, pslice, hb, use_act):
                if with_bias:
                    if use_act:
                        nc.scalar.activation(
                            h[:, pslice], acc[:, pslice], Relu, bias=b1sb[:, hb : hb + 1]
                        )
                    else:
                        nc.vector.tensor_scalar(
                            h[:, pslice],
                            acc[:, pslice],
                            b1sb[:, hb : hb + 1],
                            0.0,
                            Alu.add,
                            Alu.max,
                        )
                else:
                    if use_act:
                        nc.scalar.activation(h[:, pslice], acc[:, pslice], Relu)
                    else:
                        nc.vector.tensor_scalar_max(h[:, pslice], acc[:, pslice], 0.0)

            prev = None
            for pair in range(HB // 2):
                accs = [
                    ps1.tile([128, 2 * NC], f32, tag="acc", name="acc")
                    for _ in range(2)
                ]
                hsnap = [[None] * NSTEP for _ in range(2)]
                for s in range(NSTEP):
                    for w in range(2):
                        hb = 2 * pair + w
                        for p in range(2):
                            lhsT, rhs = mm1_ops(p, s, hb)
                            nc.tensor.matmul(
                                accs[w][:, p * NC : (p + 1) * NC],
                                lhsT,
                                rhs,
                                start=(s == 0),
                                stop=(s == NSTEP - 1),
                            )
                        h = hp.tile([128, 2 * NC], bf16, tag="h", name="h")
                        emit_relu(h, accs[w], slice(0, 2 * NC), hb, (s + w) % 2 == 0)
                        hsnap[w][s] = h
                    if prev is not None:
                        emit_mm2(prev[0], prev[1], s, evac=False)
                if prev is None:
                    init_mm2_banks()
                prev = (pair, hsnap)
            for s in range(NSTEP):
                emit_mm2(prev[0], prev[1], s, evac=True)

    nc.compile()
    return nc


def _get_program(with_bias):
    key = ("nc", with_bias)
    if key not in _CACHE:
        _CACHE[key] = _build_program(with_bias)
    return _CACHE[key]


def _run_on_hw(x, W1, b1, W2p, b2p, trace=False, trace_cores=None):
    """Run the bass kernel on 8 cores.  W2p/b2p already column-permuted so
    group g owns contiguous output columns [64g, 64g+64)."""
    from concourse.bass_utils import run_bass_kernel_spmd

    with_bias = bool(np.any(b1 != 0.0))
    nc = _get_program(with_bias)

    w2bf = np.ascontiguousarray(W2p.astype(ml_dtypes.bfloat16))
    b1t = np.ascontiguousarray(b1.reshape(HB, 128).T.astype(np.float32))
    b2t = np.ascontiguousarray(b2p.reshape(NPAIR, 128).T.astype(np.float32))
    w1c = np.ascontiguousarray(W1.astype(np.float32))

    in_maps = []
    for c in range(NCORES):
        xtc = np.ascontiguousarray(x[c * NC : (c + 1) * NC, :].T)  # (I, NC)
        in_maps.append({"xt": xtc, "w1": w1c, "w2": w2bf, "b1t": b1t, "b2t": b2t})

    kwargs = {}
    if trace:
        kwargs["trace"] = True
        if trace_cores is not None:
            kwargs["trace_cores"] = trace_cores
    res = run_bass_kernel_spmd(nc, in_maps, core_ids=list(range(NCORES)), **kwargs)

    outT = np.stack([res.results[c]["ot"] for c in range(NCORES)])  # (8, O, NC)
    out = np.ascontiguousarray(np.transpose(outT, (0, 2, 1))).reshape(N, O)
    return out, res


def _reference_numpy(x, W1, b1, W2, b2, A_mask, col_idx):
    """Exact fallback for non-canonical adjacency inputs."""
    n = x.shape[0]
    o_ = A_mask.shape[1]
    out = np.empty((n, W2.shape[1]), dtype=np.float32)
    cols_done = np.zeros(W2.shape[1], dtype=bool)
    for g in range(o_):
        cols = np.nonzero(col_idx == g)[0]
        if len(cols) == 0:
            continue
        h = np.maximum(0.0, (x * A_mask[:, g][None, :]) @ W1 + b1)
        out[:, cols] = h @ W2[:, cols] + b2[cols]
        cols_done[cols] = True
    out[:, ~cols_done] = 0.0
    return out


def kernel(x, W1, b1, W2, b2, A_mask, col_idx, _trace=False, _trace_cores=None):
    x = np.asarray(x, dtype=np.float32)
    W1 = np.asarray(W1, dtype=np.float32)
    b1 = np.asarray(b1, dtype=np.float32)
    W2 = np.asarray(W2, dtype=np.float32)
    b2 = np.asarray(b2, dtype=np.float32)
    A_mask = np.asarray(A_mask, dtype=np.float32)
    col_idx_np = np.asarray(col_idx).astype(np.int64)

    canonical = (
        x.shape == (N, I)
        and W1.shape == (I, H)
        and W2.shape == (H, O)
        and A_mask.shape == (I, O_)
        and col_idx_np.shape == (O,)
        and np.array_equal(A_mask, _canonical_mask())
        and np.all(np.bincount(col_idx_np, minlength=O_) == GROUP)
        and np.all(col_idx_np >= 0)
        and np.all(col_idx_np < O_)
    )
    if not canonical:
        return _reference_numpy(x, W1, b1, W2, b2, A_mask, col_idx_np)

    perm = np.argsort(col_idx_np, kind="stable")  # cols for group 0, then 1, ...
    W2p = W2[:, perm]
    b2p = b2[perm]
    out_p, res = _run_on_hw(x, W1, b1, W2p, b2p, trace=_trace, trace_cores=_trace_cores)
    out = np.empty_like(out_p)
    out[:, perm] = out_p
    if _trace:
        return out, res
    return out


# revision 26
# speedup vs baseline: 1.0398x; 1.0398x over previous
"""Trainium2 Bass kernel for nn_AdjacencyMaskedNet.

Reference math (N=4096, I=512, O=512, O_=8 groups, H=2048, GROUP=64):
    for each group g: h_g = relu((x * A_mask[:, g]) @ W1 + b1)
                      y_g = h_g @ W2 + b2
    out[n, j] = y_{col_idx[j]}[n, j]

With the canonical inputs, A_mask[:, g] is the prefix mask over the first
64*(g+1) inputs and col_idx = arange(512) // 64.  Structural wins:

  1. mm1 is computed *incrementally* in PSUM: a_g = a_{g-1} + (next 64-col
     block of x) @ (matching W1 rows).  One full-matmul worth of FLOPs
     instead of 8.
  2. mm2 only needs the 64 output columns of each group's y: an 8x cut.
  3. The 8 running-sum snapshots per H-block are produced by TWO chains of
     K=128 increments (K=64 LDWEIGHTS from sliced tiles measured 3x slower
     and un-hidden):
        odd  chain: B0B1 | B2B3 | B4B5 | B6B7      -> a1, a3, a5, a7
        even chain: B0 | B1B2 | B3B4 | B5B6        -> a0, a2, a4, a6
     where the even chain's pair blocks come from a 64-row-shifted DMA view
     of the same W1/x DRAM.
  4. mm2 (M=64) instructions alternate PE column halves (even groups ->
     cols 0-63, odd -> 64-127), so consecutive mm2s run concurrently in
     the array (2x), packing two groups per PSUM bank.

Sharding: data-parallel over batch, 512 rows per core, 8 cores, no
collectives.  Each core computes outT (O, 512) for its batch shard.

dtypes: mm1 in float32r (full-rate fp32, ~1.5e-4 component error), mm2 in
bf16 (h tiles are written in bf16 directly by the relu).  PSUM accumulates
fp32.
"""

import sys

if "/opt/trn_rl_repo" not in sys.path:
    sys.path.insert(0, "/opt/trn_rl_repo")

import numpy as np
import ml_dtypes

N, I, O, O_, H = 4096, 512, 512, 8, 2048
GROUP = O // O_  # 64
NCORES = 8
NC = N // NCORES  # 512 batch rows per core
HB = H // 128  # 16 H-blocks
NPAIR = O // 128  # 4 psum banks for mm2 (two 64-col groups per bank)

_CACHE = {}


def _canonical_mask():
    g = np.arange(O_)
    return (np.arange(I)[:, None] < (g[None, :] + 1) * (I // O_)).astype(np.float32)


def _build_program(with_bias):
    """Build + compile the Bass program once per process."""
    import concourse.tile as tile
    from concourse import bacc, mybir

    f32 = mybir.dt.float32
    f32r = mybir.dt.float32r
    bf16 = mybir.dt.bfloat16
    Relu = mybir.ActivationFunctionType.Relu
    Ident = mybir.ActivationFunctionType.Identity
    Alu = mybir.AluOpType

    nc = bacc.Bacc("TRN2", target_bir_lowering=False, debug=False, num_devices=NCORES)

    xt = nc.dram_tensor("xt", [I, NC], f32, kind="ExternalInput").ap()
    w1 = nc.dram_tensor("w1", [I, H], f32, kind="ExternalInput").ap()
    w2 = nc.dram_tensor("w2", [H, O], bf16, kind="ExternalInput").ap()
    b1t = nc.dram_tensor("b1t", [128, HB], f32, kind="ExternalInput").ap()
    b2t = nc.dram_tensor("b2t", [128, NPAIR], f32, kind="ExternalInput").ap()
    ot = nc.dram_tensor("ot", [O, NC], f32, kind="ExternalOutput").ap()

    with tile.TileContext(nc) as tc:
        with (
            tc.tile_pool(name="const", bufs=1) as cp,
            tc.tile_pool(name="hpool", bufs=12) as hp,
            tc.tile_pool(name="opool", bufs=1) as op,
            tc.tile_pool(name="ps1", bufs=2, space="PSUM") as ps1,
            tc.tile_pool(name="ps2", bufs=1, space="PSUM") as ps2,
        ):
            # ---- tiles for resident inputs ----
            xnat = [cp.tile([128, NC], f32r, name=f"xn{m}") for m in range(4)]
            xsh = [cp.tile([128, NC], f32r, name=f"xh{m}") for m in range(3)]
            w1nat = [
                [cp.tile([128, 512], f32r, name=f"w1n{m}_{q}") for q in range(4)]
                for m in range(4)
            ]
            w1sh = [
                [cp.tile([128, 512], f32r, name=f"w1s{m}_{q}") for q in range(4)]
                for m in range(3)
            ]
            w2sb = [cp.tile([128, O], bf16, name=f"w2_{k}") for k in range(HB)]
            b1sb = cp.tile([128, HB], f32, name="b1sb")
            b2sb = cp.tile([128, NPAIR], f32, name="b2sb")

            # ---- DMAs in first-use order (all on the sync HWDGE ring):
            # x block then its step-matching W1 chunk, so chain (hb0, s=0)
            # can start after the first two transfers.
            def dma_w1n(m, q):
                cols = slice(q * 512, (q + 1) * 512)
                nc.sync.dma_start(
                    w1nat[m][q][:], w1[m * 128 : (m + 1) * 128, cols].bitcast(f32r)
                )

            def dma_w1s(m, q):
                cols = slice(q * 512, (q + 1) * 512)
                nc.sync.dma_start(
                    w1sh[m][q][:],
                    w1[64 + m * 128 : 64 + (m + 1) * 128, cols].bitcast(f32r),
                )

            nc.sync.dma_start(xnat[0][:], xt[0:128, :].bitcast(f32r))
            dma_w1n(0, 0)
            nc.sync.dma_start(b1sb[:], b1t[:])
            nc.sync.dma_start(b2sb[:], b2t[:])
            for m in range(1, 4):
                nc.sync.dma_start(xnat[m][:], xt[m * 128 : (m + 1) * 128, :].bitcast(f32r))
                dma_w1n(m, 0)
                nc.sync.dma_start(
                    xsh[m - 1][:],
                    xt[64 + (m - 1) * 128 : 64 + m * 128, :].bitcast(f32r),
                )
                dma_w1s(m - 1, 0)
            for q in range(4):
                if q > 0:
                    for m in range(4):
                        dma_w1n(m, q)
                    for m in range(3):
                        dma_w1s(m, q)
                for k in range(q * 4, (q + 1) * 4):
                    nc.sync.dma_start(w2sb[k][:], w2[k * 128 : (k + 1) * 128, :])

            # zero operands for the has_written-initializing dummy matmuls
            z1 = cp.tile([1, 128], bf16, name="z1")
            nc.gpsimd.memset(z1[:], 0.0)
            z2 = cp.tile([1, NC], bf16, name="z2")
            nc.gpsimd.memset(z2[:], 0.0)

            # ---- mm2 accumulators: 4 banks, two 64-col groups per bank.
            # start=True on any matmul clears has_written for the WHOLE bank
            # (nuking the other group), so init each bank once with a zero
            # matmul and accumulate with start=False afterwards.
            mm2ps = [ps2.tile([128, NC], f32, name=f"mm2_{t}") for t in range(NPAIR)]

            def init_mm2_banks():
                # init has_written for the whole of each mm2 bank once, so the
                # two packed groups can both accumulate with start=False
                for t in range(NPAIR):
                    nc.tensor.matmul(
                        mm2ps[t][:], z1[:], z2[:], start=True, stop=False,
                        skip_group_check=True,
                    )

            # mm1 increment operands for (parity, step, hb-quad):
            #   odd  chain step s: lhsT = w1nat[s][q] slice, rhs = xnat[s]
            #   even chain step 0: lhsT = w1nat[0][q][0:64] slice, rhs = xnat[0][0:64]
            #   even chain step s>0: lhsT = w1sh[s-1][q] slice, rhs = xsh[s-1]
            def mm1_ops(parity, s, hb):
                q, hq = hb // 4, hb % 4
                colsl = slice(hq * 128, (hq + 1) * 128)
                if parity == 1:
                    return w1nat[s][q][:, colsl], xnat[s][:]
                if s == 0:
                    return w1nat[0][q][0:64, colsl], xnat[0][0:64, :]
                return w1sh[s - 1][q][:, colsl], xsh[s - 1][:]

            # ---- main loop: hb pairs, 4 chains in flight:
            #   c0=(hbA, even) c1=(hbA, odd) c2=(hbB, even) c3=(hbB, odd)
            # mm2 runs one step behind mm1 (software pipelining) so the PE
            # never head-of-line blocks on a relu.  Chain (parity p, step s)
            # snapshots group g = 2s + p; mm2 for group g targets psum bank
            # s, column half p -- consecutive mm2s alternate PE column
            # halves and run concurrently (M=64 col tiling).
            # Each hb's even+odd chains share a double-width (128, 1024)
            # accumulator (2 banks; even chain in cols 0-511, odd in
            # 512-1023): ONE wide relu per (hb, step) snapshots both,
            # halving the relu op count.  Both halves share hb, so the
            # per-partition b1 bias column still applies in bias mode.
            # The fp32r<->bf16 dtype switch on the PE costs ~100-250ns, so
            # all 16 mm1s of an hb pair are batched, then all 16 mm2s
            # (measured 180ns/MM at batch=16 vs 244 at batch=4).  The relus
            # overlap the mm1 batch; the mm2 batch overlaps the next pair's
            # weight DMAs and relu completions.
            # PE stream per step: [4x mm1(pair P, s)] [4x mm2(pair P-1, s)].
            # The mm2 batch of the PREVIOUS pair gives the PE ~8 matmuls of
            # ready work between the dependent mm1(s) -> relu(s) -> mm1(s+1)
            # links, hiding the ~1.2us wide-relu latency.
            NSTEP = 4

            def emit_mm2(pair, hsnap, s, evac):
                for w in range(2):
                    hb = 2 * pair + w
                    for p in range(2):
                        g = 2 * s + p
                        nc.tensor.matmul(
                            mm2ps[s][64 * p : 64 * p + 64, :],
                            w2sb[hb][:, g * GROUP : (g + 1) * GROUP],
                            hsnap[w][s][:, p * NC : (p + 1) * NC],
                            start=False,
                            stop=(hb == HB - 1 and p == 1),
                            skip_group_check=True,
                        )
                if evac:
                    os_ = op.tile([128, NC], f32, name=f"os{s}")
                    if s % 2 == 0:
                        nc.scalar.activation(
                            os_[:], mm2ps[s][:], Ident, bias=b2sb[:, s : s + 1]
                        )
                    else:
                        nc.vector.tensor_scalar_add(os_[:], mm2ps[s][:], b2sb[:, s : s + 1])
                    nc.sync.dma_start(ot[s * 128 : (s + 1) * 128, :], os_[:])

            def emit_relu(h, acc, pslice, hb, use_act):
                if with_bias:
                    if use_act:
                        nc.scalar.activation(
                            h[:, pslice], acc[:, pslice], Relu, bias=b1sb[:, hb : hb + 1]
                        )
                    else:
                        nc.vector.tensor_scalar(
                            h[:, pslice],
                            acc[:, pslice],
                            b1sb[:, hb : hb + 1],
                            0.0,
                            Alu.add,
                            Alu.max,
                        )
                else:
                    if use_act:
                        nc.scalar.activation(h[:, pslice], acc[:, pslice], Relu)
                    else:
                        nc.vector.tensor_scalar_max(h[:, pslice], acc[:, pslice], 0.0)

            prev = None
            for pair in range(HB // 2):
                accs = [
                    ps1.tile([128, 2 * NC], f32, tag="acc", name="acc")
                    for _ in range(2)
                ]
                hsnap = [[None] * NSTEP for _ in range(2)]
                for s in range(NSTEP):
                    for w in range(2):
                        hb = 2 * pair + w
                        for p in range(2):
                            lhsT, rhs = mm1_ops(p, s, hb)
                            nc.tensor.matmul(
                                accs[w][:, p * NC : (p + 1) * NC],
                                lhsT,
                                rhs,
                                start=(s == 0),
                                stop=(s == NSTEP - 1),
                            )
                        h = hp.tile([128, 2 * NC], bf16, tag="h", name="h")
                        emit_relu(h, accs[w], slice(0, 2 * NC), hb, (s + w) % 2 == 0)
                        hsnap[w][s] = h
                    if prev is not None:
                        emit_mm2(prev[0], prev[1], s, evac=False)
                if prev is None:
                    init_mm2_banks()
                prev = (pair, hsnap)
            for s in range(NSTEP):
                emit_mm2(prev[0], prev[1], s, evac=True)

    nc.compile()
    return nc


def _get_program(with_bias):
    key = ("nc", with_bias)
    if key not in _CACHE:
        _CACHE[key] = _build_program(with_bias)
    return _CACHE[key]


def _run_on_hw(x, W1, b1, W2p, b2p, trace=False, trace_cores=None):
    """Run the bass kernel on 8 cores.  W2p/b2p already column-permuted so
    group g owns contiguous output columns [64g, 64g+64)."""
    from concourse.bass_utils import run_bass_kernel_spmd

    with_bias = bool(np.any(b1 != 0.0))
    nc = _get_program(with_bias)

    w2bf = np.ascontiguousarray(W2p.astype(ml_dtypes.bfloat16))
    b1t = np.ascontiguousarray(b1.reshape(HB, 128).T.astype(np.float32))
    b2t = np.ascontiguousarray(b2p.reshape(NPAIR, 128).T.astype(np.float32))
    w1c = np.ascontiguousarray(W1.astype(np.float32))

    in_maps = []
    for c in range(NCORES):
        xtc = np.ascontiguousarray(x[c * NC : (c + 1) * NC, :].T)  # (I, NC)
        in_maps.append({"xt": xtc, "w1": w1c, "w2": w2bf, "b1t": b1t, "b2t": b2t})

    kwargs = {}
    if trace:
        kwargs["trace"] = True
        if trace_cores is not None:
            kwargs["trace_cores"] = trace_cores
    res = run_bass_kernel_spmd(nc, in_maps, core_ids=list(range(NCORES)), **kwargs)

    outT = np.stack([res.results[c]["ot"] for c in range(NCORES)])  # (8, O, NC)
    out = np.ascontiguousarray(np.transpose(outT, (0, 2, 1))).reshape(N, O)
    return out, res


def _reference_numpy(x, W1, b1, W2, b2, A_mask, col_idx):
    """Exact fallback for non-canonical adjacency inputs."""
    n = x.shape[0]
    o_ = A_mask.shape[1]
    out = np.empty((n, W2.shape[1]), dtype=np.float32)
    cols_done = np.zeros(W2.shape[1], dtype=bool)
    for g in range(o_):
        cols = np.nonzero(col_idx == g)[0]
        if len(cols) == 0:
            continue
        h = np.maximum(0.0, (x * A_mask[:, g][None, :]) @ W1 + b1)
        out[:, cols] = h @ W2[:, cols] + b2[cols]
        cols_done[cols] = True
    out[:, ~cols_done] = 0.0
    return out


def kernel(x, W1, b1, W2, b2, A_mask, col_idx, _trace=False, _trace_cores=None):
    x = np.asarray(x, dtype=np.float32)
    W1 = np.asarray(W1, dtype=np.float32)
    b1 = np.asarray(b1, dtype=np.float32)
    W2 = np.asarray(W2, dtype=np.float32)
    b2 = np.asarray(b2, dtype=np.float32)
    A_mask = np.asarray(A_mask, dtype=np.float32)
    col_idx_np = np.asarray(col_idx).astype(np.int64)

    canonical = (
        x.shape == (N, I)
        and W1.shape == (I, H)
        and W2.shape == (H, O)
        and A_mask.shape == (I, O_)
        and col_idx_np.shape == (O,)
        and np.array_equal(A_mask, _canonical_mask())
        and np.all(np.bincount(col_idx_np, minlength=O_) == GROUP)
        and np.all(col_idx_np >= 0)
        and np.all(col_idx_np < O_)
    )
    if not canonical:
        return _reference_numpy(x, W1, b1, W2, b2, A_mask, col_idx_np)

    perm = np.argsort(col_idx_np, kind="stable")  # cols for group 0, then 1, ...
    W2p = W2[:, perm]
    b2p = b2[perm]
    out_p, res = _run_on_hw(x, W1, b1, W2p, b2p, trace=_trace, trace_cores=_trace_cores)
    out = np.empty_like(out_p)
    out[:, perm] = out_p
    if _trace:
        return out, res
    return out
